# revision 18
# baseline (speedup 1.0000x reference)
"""Trainium2 Bass kernel for nn_AttentionDist (mlp-scored attention).

Per-core work (data-parallel over batch B=8 across 8 NeuronCores):
  queries [128,256], context [256,256], labels [256] -> per-batch outputs.

Math (per batch):
  ctx_pre = context @ Wc + onehot(labels) @ (label_emb @ Wl) + b1      [cs,H]
  q_pre   = queries @ Wq                                              [n,H]
  attn[n,c] = relu(q_pre[n,:] + ctx_pre[c,:]) @ W2 + b2               [n,cs]
  P = exp(attn);  Z = P.sum(-1)
  context_vector = (P @ context) / Z
  l_softmax_classes = log((P @ onehot) / Z)

Device strategy:
  * ctx_pre / q_pre are computed transposed ([H, cs] / [H, n]) so the relu
    broadcast-add runs as one fused per-partition-bias op per query n:
        h_n[h,c] = relu(ctx_preT[h,c] + q_preT[h,n])
    produced round-robin on VectorE (bf16 4x mode) and ScalarE.
  * attn rows are accumulated in PSUM via a "sliding-window" masked
    stationary: Zt[:,128-n:256-n] has W2 in column n and zeros elsewhere,
    so matmul n adds attn[n,:] into PSUM row n only. Two PSUM halves let
    the first half's softmax (exp + transposes) overlap the second half.
  * All inputs ride in one packed [128, 2096] f32 tensor whose blocks are
    pre-laid-out by the host (context/queries also in transposed layout --
    pure data movement), loaded by five DMAs ordered by consumer.
  * Prologue/epilogue matmuls run on float32r views (full-rate for >=256
    moving columns); only the main loop uses bf16.
  * One ACT table load total (natural_log_exp_and_others covers
    relu/exp/ln/copy); outputs leave via two DMAs (attn+cv, then lsc).
  * Softmax skips the max-subtraction (attn is O(1), exp is safe in f32).
"""

import os
import sys

for _p in ("/opt/trn_rl_repo", "/root/.axon_site", "/root/.axon_site/_ro/trn_rl_repo"):
    if os.path.isdir(_p) and _p not in sys.path:
        sys.path.append(_p)

import numpy as np

import concourse.bacc as bacc
import concourse.tile as tile
from concourse import mybir
from concourse.bass_utils import run_bass_kernel_spmd

B, N, CS, DIM, CLD, K, H = 8, 128, 256, 256, 32, 21, 128
F32 = mybir.dt.float32
F32R = mybir.dt.float32r
BF16 = mybir.dt.bfloat16
AF = mybir.ActivationFunctionType
OP = mybir.AluOpType

# relu-tile producer pattern, cycled per query index n
PRODUCERS = ("v", "v", "v", "a")

# ---- packed input column layout (bigin [128, 2096] f32) ----
# S1 (sync, bus slot 1): extras + label_embT + identity
C_B1 = 0
C_W2 = 1
C_B2 = 2
C_IOC = 3           # iota col, rows 0:21
C_LAB2 = 4          # labels as [128,2]
C_IO21 = 6          # iota row 0..20        [128,21]
C_LET = 27          # label_embT rows 0:32  [32,21]
C_WL = 48           # W1[512:544] rows 0:32 [32,128]
C_ID = 176          # identity              [128,128]
S1_END = 304
# S2 (scalar-issued, bus slot 2): contextT
C_CT0 = 304         # contextT rows d=0:128   [128,256]
C_CT1 = 560         # contextT rows d=128:256 [128,256]
S2_END = 816
# S3 (sync, bus slot 3): Wc
C_WC0 = 816
C_WC1 = 944
S3_END = 1072
# S4 (sync, bus slot 4): queriesT + Wq
C_QT0 = 1072        # queriesT rows d=0:128   [128,128]
C_QT1 = 1200        # queriesT rows d=128:256 [128,128]
C_WQ0 = 1328
C_WQ1 = 1456
S4_END = 1584
# S5 (sync, bus slot 5, epilogue-only): natural context
C_CF0 = 1584        # context rows 0:128    [128,256]
C_CF1 = 1840        # context rows 128:256  [128,256]
IN_W = 2096

# ---- packed output column layout (out_all [128, 544] f32) ----
O_ATTN = 0
O_CV = 256
O_LSC = 512
OUT_W = 544


def _patch_act_tables():
    """Make Bacc's act-table-load pass pick natural_log_exp_and_others for
    every function we use (it genuinely contains exp/ln/relu/copy/identity),
    so the kernel pays exactly one ACT table load."""
    import concourse.bacc as bacc_mod

    real = bacc_mod.get_activation_tables
    if getattr(real, "_attn_patched", False):
        return

    def patched(arch):
        tabs = real(arch)
        want = tabs.get("natural_log_exp_and_others", set())
        need = {AF.Exp, AF.Ln, AF.Relu, AF.Copy, AF.Identity}
        if not need.issubset(want):
            return tabs  # unexpected act_info: fall back to default sets
        return {
            nm: (fns if nm == "natural_log_exp_and_others" else set())
            for nm, fns in tabs.items()
        }

    patched._attn_patched = True
    bacc_mod.get_activation_tables = patched


def build_nc():
    _patch_act_tables()
    nc = bacc.Bacc("TRN2", target_bir_lowering=False, debug=False)

    in_e = nc.dram_tensor("bigin", [128, IN_W], F32, kind="ExternalInput").ap()
    out_e = nc.dram_tensor("out_all", [N, OUT_W], F32, kind="ExternalOutput").ap()

    def r(ap):
        # FP32R needs producer-side rounding (BIR verifier rejects raw DMA
        # data feeding an fp32r matmul), so run prologue/epilogue matmuls
        # in plain fp32.
        return ap

    with tile.TileContext(nc) as tc:
        with (
            tc.tile_pool(name="const", bufs=1) as const,
            tc.tile_pool(name="work", bufs=2) as work,
            tc.tile_pool(name="hp", bufs=12) as hp,
            tc.tile_pool(name="ps", bufs=3, space="PSUM") as ps,
            tc.tile_pool(name="psA", bufs=1, space="PSUM") as psA,
        ):
            dma = nc.sync.dma_start

            # ---- packed input load: 5 DMAs ordered by consumer ------------
            # Bus order: sync#1, scalar#1, sync#2, sync#3, sync#4.
            big = const.tile([128, IN_W], F32)
            dma(out=big[:, 0:S1_END], in_=in_e[:, 0:S1_END])
            nc.scalar.dma_start(
                out=big[:, S1_END:S2_END], in_=in_e[:, S1_END:S2_END]
            )
            dma(out=big[:, S2_END:S3_END], in_=in_e[:, S2_END:S3_END])
            dma(out=big[:, S3_END:S4_END], in_=in_e[:, S3_END:S4_END])
            dma(out=big[:, S4_END:IN_W], in_=in_e[:, S4_END:IN_W])

            b1c = big[:, C_B1 : C_B1 + 1]
            w2f = big[:, C_W2 : C_W2 + 1]
            b2c = big[:, C_B2 : C_B2 + 1]
            io21c = big[0:K, C_IOC : C_IOC + 1]
            lab2 = big[:, C_LAB2 : C_LAB2 + 2]
            io21 = big[:, C_IO21 : C_IO21 + K]
            leT = big[0:CLD, C_LET : C_LET + K]
            idf = big[:, C_ID : C_ID + 128]
            cT0 = big[:, C_CT0 : C_CT0 + 256]
            cT1 = big[:, C_CT1 : C_CT1 + 256]
            wc0 = big[:, C_WC0 : C_WC0 + 128]
            wc1 = big[:, C_WC1 : C_WC1 + 128]
            wlf = big[0:CLD, C_WL : C_WL + 128]
            qT0 = big[:, C_QT0 : C_QT0 + 128]
            qT1 = big[:, C_QT1 : C_QT1 + 128]
            wq0 = big[:, C_WQ0 : C_WQ0 + 128]
            wq1 = big[:, C_WQ1 : C_WQ1 + 128]
            cf0 = big[:, C_CF0 : C_CF0 + 256]
            cf1 = big[:, C_CF1 : C_CF1 + 256]

            # PE warmup: dummy matmuls until the input DMA lands, so the
            # HAM clock-gate is released before real PE work
            warm = const.tile([128, 256], BF16)
            nc.vector.memset(warm, 0.0)
            wps = ps.tile([128, 256], F32, tag="ps")
            for _ in range(12):
                nc.tensor.matmul(
                    wps, lhsT=warm[:, 0:128], rhs=warm, start=True, stop=True
                )

            # ---- sliding-window stationary: zeros with W2 at column 128 ---
            Zt = const.tile([128, 2 * 128], BF16)
            nc.vector.memset(Zt, 0.0)
            nc.vector.tensor_copy(Zt[:, 128:129], w2f)

            # ---- one-hot encodings (f32); onehotT via PE transposes -------
            oh0 = const.tile([128, K], F32)  # onehot[c,k], c=0:128
            nc.vector.tensor_scalar(oh0, io21, lab2[:, 0:1], None, op0=OP.is_equal)
            oh1 = const.tile([128, K], F32)  # onehot[c,k], c=128:256
            nc.vector.tensor_scalar(oh1, io21, lab2[:, 1:2], None, op0=OP.is_equal)
            ohT = const.tile([K, CS], F32)  # onehotT[k,c]
            to0 = ps.tile([K, 128], F32, tag="pst")
            nc.tensor.transpose(to0, r(oh0), r(idf))
            nc.vector.tensor_copy(ohT[:, 0:128], to0.bitcast(F32))
            to1 = ps.tile([K, 128], F32, tag="pst")
            nc.tensor.transpose(to1, r(oh1), r(idf))
            nc.vector.tensor_copy(ohT[:, 128:256], to1.bitcast(F32))

            # ---- Wle = label_emb @ Wl  [K, H] ----------------------------
            pw = ps.tile([K, H], F32, tag="ps")
            nc.tensor.matmul(pw, lhsT=r(leT), rhs=r(wlf), start=True, stop=True)
            wle = const.tile([K, H], F32)
            nc.scalar.copy(wle, pw)

            # ---- ctx_preT = WcT @ contextT + WleT @ onehotT + b1  [H, CS] -
            pc = ps.tile([128, CS], F32, tag="ps")
            nc.tensor.matmul(pc, lhsT=r(wc0), rhs=r(cT0), start=True, stop=False)
            nc.tensor.matmul(pc, lhsT=r(wc1), rhs=r(cT1), start=False, stop=False)
            nc.tensor.matmul(pc, lhsT=r(wle), rhs=r(ohT), start=False, stop=True)
            ctxpre_b = const.tile([H, CS], BF16)
            nc.vector.tensor_scalar(ctxpre_b, pc, b1c, None, op0=OP.add)

            # ---- q_preT = WqT @ queriesT  [H, N], in two n-halves ---------
            qpre_f = const.tile([H, N], F32)
            for lo in (0, 64):
                pq = ps.tile([128, 64], F32, tag="pst")
                nc.tensor.matmul(
                    pq, lhsT=r(wq0), rhs=r(qT0[:, lo : lo + 64]),
                    start=True, stop=False,
                )
                nc.tensor.matmul(
                    pq, lhsT=r(wq1), rhs=r(qT1[:, lo : lo + 64]),
                    start=False, stop=True,
                )
                nc.scalar.copy(qpre_f[:, lo : lo + 64], pq)

            # ---- main loop: attn[n,:] = relu(ctx_preT + q_preT[:,n]) @ W2
            # Two PSUM halves; first half's softmax overlaps second half.
            attn_psA = psA.tile([128, CS], F32, tag="psA")
            attn_psB = psA.tile([128, CS], F32, tag="psB")
            out_t = work.tile([N, OUT_W], F32)
            P = work.tile([N, CS], F32)
            pt0 = work.tile([128, 128], F32)
            pt1 = work.tile([128, 128], F32)
            NH = N // 2

            def half_softmax(lo):
                hi = lo + NH
                src_ps = attn_psA if lo == 0 else attn_psB
                nc.scalar.activation(P[lo:hi, :], src_ps[lo:hi, :], AF.Exp)
                nc.vector.tensor_scalar(
                    out_t[lo:hi, O_ATTN : O_ATTN + CS],
                    src_ps[lo:hi, :],
                    b2c[lo:hi, :],
                    None,
                    op0=OP.add,
                )
                dma(
                    out=out_e[lo:hi, O_ATTN : O_ATTN + CS],
                    in_=out_t[lo:hi, O_ATTN : O_ATTN + CS],
                )
                for cchunk, pt in ((0, pt0), (1, pt1)):
                    tph = ps.tile([128, NH], F32, tag="pst")
                    nc.tensor.transpose(
                        tph,
                        r(P[lo:hi, cchunk * 128 : (cchunk + 1) * 128]),
                        r(idf[lo:hi, lo:hi]),
                    )
                    nc.vector.tensor_copy(pt[:, lo:hi], tph.bitcast(F32))

            for n in range(N):
                h = hp.tile([128, CS], BF16, tag="h")
                eng = PRODUCERS[n % len(PRODUCERS)]
                if eng == "a":
                    nc.scalar.activation(
                        h, ctxpre_b, AF.Relu, bias=qpre_f[:, n : n + 1]
                    )
                else:
                    nc.vector.tensor_scalar(
                        h, ctxpre_b, qpre_f[:, n : n + 1], 0.0, op0=OP.add, op1=OP.max
                    )
                nc.tensor.matmul(
                    attn_psA if n < NH else attn_psB,
                    lhsT=Zt[:, 128 - n : 256 - n],
                    rhs=h,
                    start=(n == 0 or n == NH),
                    stop=(n == NH - 1 or n == N - 1),
                )
                if n == NH - 1:
                    half_softmax(0)
            half_softmax(NH)

            # ---- epilogue ------------------------------------------------
            cv_ps = ps.tile([128, DIM], F32, tag="ps")
            nc.tensor.matmul(cv_ps, lhsT=r(pt0), rhs=r(cf0), start=True, stop=False)
            nc.tensor.matmul(cv_ps, lhsT=r(pt1), rhs=r(cf1), start=False, stop=True)
            s_ps = ps.tile([128, K], F32, tag="ps")
            nc.tensor.matmul(s_ps, lhsT=r(pt0), rhs=r(oh0), start=True, stop=False)
            nc.tensor.matmul(s_ps, lhsT=r(pt1), rhs=r(oh1), start=False, stop=True)

            z = work.tile([128, 1], F32)
            nc.vector.tensor_reduce(z, s_ps, axis=mybir.AxisListType.X, op=OP.add)
            zr = work.tile([128, 1], F32)
            nc.vector.reciprocal(zr, z)
            srel = work.tile([128, K], F32)
            nc.vector.tensor_scalar(srel, s_ps, zr, None, op0=OP.mult)
            nc.scalar.activation(out_t[:, O_LSC : O_LSC + K], srel, AF.Ln)
            nc.vector.tensor_scalar(
                out_t[:, O_CV : O_CV + DIM], cv_ps, zr, None, op0=OP.mult
            )
            dma(out=out_e[:, O_CV : O_CV + DIM], in_=out_t[:, O_CV : O_CV + DIM])
            nc.scalar.dma_start(
                out=out_e[:, O_LSC : O_LSC + K], in_=out_t[:, O_LSC : O_LSC + K]
            )

    nc.compile()
    return nc


_NC = None


def get_nc():
    global _NC
    if _NC is None:
        _NC = build_nc()
    return _NC


def make_in_maps(queries, context, context_labels, label_emb, W1, b1, W2, b2):
    queries = np.asarray(queries, dtype=np.float32)
    context = np.asarray(context, dtype=np.float32)
    labels = np.asarray(context_labels).astype(np.float32)
    label_emb = np.asarray(label_emb, dtype=np.float32)
    W1 = np.asarray(W1, dtype=np.float32)
    b1 = np.asarray(b1, dtype=np.float32).reshape(H)
    W2 = np.asarray(W2, dtype=np.float32).reshape(H)
    b2f = float(np.asarray(b2, dtype=np.float32).reshape(-1)[0])

    base = np.zeros((128, IN_W), dtype=np.float32)
    base[:, C_B1] = b1
    base[:, C_W2] = W2
    base[:, C_B2] = b2f
    base[0:K, C_IOC] = np.arange(K, dtype=np.float32)
    base[:, C_IO21 : C_IO21 + K] = np.arange(K, dtype=np.float32)[None, :]
    base[0:CLD, C_LET : C_LET + K] = label_emb.T
    base[:, C_ID : C_ID + 128] = np.eye(128, dtype=np.float32)
    base[:, C_WC0 : C_WC0 + 128] = W1[256:384]
    base[:, C_WC1 : C_WC1 + 128] = W1[384:512]
    base[0:CLD, C_WL : C_WL + 128] = W1[512:544]
    base[:, C_WQ0 : C_WQ0 + 128] = W1[0:128]
    base[:, C_WQ1 : C_WQ1 + 128] = W1[128:256]

    in_maps = []
    for b in range(B):
        m = base.copy()
        ctxT = context[b].T  # [DIM, CS] -- layout prep only
        m[:, C_CT0 : C_CT0 + 256] = ctxT[0:128]
        m[:, C_CT1 : C_CT1 + 256] = ctxT[128:256]
        qT = queries[b].T  # [DIM, N]
        m[:, C_QT0 : C_QT0 + 128] = qT[0:128]
        m[:, C_QT1 : C_QT1 + 128] = qT[128:256]
        m[:, C_CF0 : C_CF0 + 256] = context[b, 0:128]
        m[:, C_CF1 : C_CF1 + 256] = context[b, 128:256]
        lab = labels[b]
        m[:, C_LAB2] = lab[0:128]
        m[:, C_LAB2 + 1] = lab[128:256]
        in_maps.append({"bigin": m})
    return in_maps


def kernel(queries, context, context_labels, mask, label_emb, W1, b1, W2, b2):
    nc = get_nc()
    in_maps = make_in_maps(
        queries, context, context_labels, label_emb, W1, b1, W2, b2
    )
    res = run_bass_kernel_spmd(nc, in_maps, core_ids=list(range(B))).results
    out = np.stack([res[b]["out_all"] for b in range(B)])  # [B, 128, 544]
    cv = np.ascontiguousarray(out[:, :, O_CV : O_CV + DIM], dtype=np.float32)
    lsc = np.ascontiguousarray(out[:, :, O_LSC : O_LSC + K], dtype=np.float32)
    attn = np.ascontiguousarray(out[:, :, O_ATTN : O_ATTN + CS], dtype=np.float32)
    return cv, lsc, attn
